# revision 5
# baseline (speedup 1.0000x reference)
"""Trainium2 Bass kernel for the autoregressive LSTM (data-parallel over batch, 8 cores).

Per core (batch shard BL=64), per time step:
  gates[64, 4096] = vecT.T @ Wk  (vecT = [xT; zT; hT] feature-major bf16 lhsT chunks,
                                  Wk = [W_ih | W_hh].T bf16 rhs resident in SBUF)
  + bias via K=1 ones-matmul; cell elementwise in fp32 batch-major;
  h_new transposed back to feature-major via PE transposes (bf16 copies);
  z = tanh(h_new @ W_fc.T + b_fc) * mask  computed batch-major, transposed for feedback.
"""

import numpy as np

T_FULL, B_FULL, I_DIM, H_DIM, O_DIM = 512, 512, 128, 1024, 128
NCORES = 8
BL = B_FULL // NCORES  # 64
KCH = 10               # contraction chunks of 128: [x, z, h0..h7]
G = 4 * H_DIM          # 4096

_cache = {}


def _build(T_steps, U):
    """Build the Bass program. Returns nc."""
    import concourse.bass as bass
    import concourse.mybir as mybir
    import concourse.tile as tile
    from concourse import bacc
    from concourse.bass import ds
    from concourse.masks import make_identity

    dt = mybir.dt
    AF = mybir.ActivationFunctionType
    f32, bf16 = dt.float32, dt.bfloat16

    assert T_steps % U == 0 and U % 2 == 0

    nc = bacc.Bacc("TRN2", target_bir_lowering=False, debug=False)
    xT_d = nc.dram_tensor("xT", [T_steps, I_DIM, BL], bf16, kind="ExternalInput")
    h0T_d = nc.dram_tensor("h0T", [8, 128, BL], bf16, kind="ExternalInput")
    z0T_d = nc.dram_tensor("z0T", [128, BL], bf16, kind="ExternalInput")
    c0_d = nc.dram_tensor("c0", [BL, H_DIM], f32, kind="ExternalInput")
    mask_d = nc.dram_tensor("masks", [BL, T_steps], f32, kind="ExternalInput")
    Wk_d = nc.dram_tensor("Wk", [KCH, 128, G], bf16, kind="ExternalInput")
    bias_d = nc.dram_tensor("bias", [1, G], bf16, kind="ExternalInput")
    WfcT_d = nc.dram_tensor("WfcT", [8, 128, O_DIM], bf16, kind="ExternalInput")
    bfc_d = nc.dram_tensor("bfc", [1, O_DIM], bf16, kind="ExternalInput")
    oz_d = nc.dram_tensor("out_z", [T_steps, BL, O_DIM], f32, kind="ExternalOutput")
    oh_d = nc.dram_tensor("out_h", [BL, H_DIM], f32, kind="ExternalOutput")
    oc_d = nc.dram_tensor("out_c", [BL, H_DIM], f32, kind="ExternalOutput")

    with tile.TileContext(nc) as tc:
        with (
            tc.tile_pool(name="state", bufs=1) as sp,
            tc.tile_pool(name="work", bufs=2) as wp,
            tc.tile_pool(name="xin", bufs=2) as xp,
            tc.tile_pool(name="gp", bufs=3, space="PSUM") as gp,
            tc.tile_pool(name="tp", bufs=1, space="PSUM") as tp,
            tc.tile_pool(name="zp", bufs=1, space="PSUM") as zp,
        ):
            # ---- resident tensors ----
            Wk_sb = [sp.tile([128, G], bf16, tag=f"wk{k}", name=f"wk{k}") for k in range(KCH)]
            for k in range(KCH):
                nc.sync.dma_start(Wk_sb[k][:], Wk_d[k])
            bias_sb = sp.tile([1, G], bf16, tag="bias", name="bias")
            nc.sync.dma_start(bias_sb[:], bias_d[:])
            WfcT_sb = [sp.tile([128, O_DIM], bf16, tag=f"wfc{k}", name=f"wfc{k}") for k in range(8)]
            for k in range(8):
                nc.sync.dma_start(WfcT_sb[k][:], WfcT_d[k])
            bfc_sb = sp.tile([1, O_DIM], bf16, tag="bfc", name="bfc")
            nc.sync.dma_start(bfc_sb[:], bfc_d[:])
            ones_sb = sp.tile([1, BL], bf16, tag="ones", name="ones")
            nc.gpsimd.memset(ones_sb[:], 1.0)
            ident = sp.tile([BL, BL], f32, tag="ident", name="ident")
            make_identity(nc, ident[:])

            # ---- ping-pong state ----
            hT = [[sp.tile([128, BL], bf16, tag=f"h{p}_{k}", name=f"h{p}_{k}") for k in range(8)]
                  for p in range(2)]
            zT = [sp.tile([128, BL], bf16, tag=f"z{p}", name=f"z{p}") for p in range(2)]
            cst = [sp.tile([BL, H_DIM], f32, tag=f"c{p}", name=f"c{p}") for p in range(2)]
            nc.sync.dma_start(zT[0][:], z0T_d[:])
            nc.sync.dma_start(cst[0][:], c0_d[:])
            for k in range(8):
                nc.sync.dma_start(hT[0][k][:], h0T_d[k])

            ozr = oz_d.rearrange("t b o -> b t o")
            xr = xT_d.rearrange("t p b -> p t b")

            final_h = [None]

            def step(j, x_tile, m_tile, zstage):
                """One LSTM step. Reads state slot j%2, writes slot (j+1)%2."""
                rd, wr = j % 2, (j + 1) % 2
                vecT = [x_tile, zT[rd]] + hT[rd]

                # gates matmuls: group A = {i, g}, group B = {f, o}
                # gate base columns in G: i=0, f=1024, g=2048, o=3072
                gtiles = {}
                for grp in (("i", "g"), ("f", "o")):
                    for gname in grp:
                        gtiles[gname] = gp.tile([BL, H_DIM], f32, tag="gpsum", name="gpsum")
                    gbase = {"i": 0, "f": 1024, "g": 2048, "o": 3072}
                    for half in range(2):
                        for gname in grp:
                            col0 = gbase[gname] + half * 512
                            nc.tensor.matmul(
                                gtiles[gname][:, half * 512:(half + 1) * 512],
                                ones_sb[:],
                                bias_sb[:, col0:col0 + 512],
                                start=True, stop=False,
                            )
                    for k in range(KCH):
                        for gname in grp:
                            for half in range(2):
                                col0 = gbase[gname] + half * 512
                                nc.tensor.matmul(
                                    gtiles[gname][:, half * 512:(half + 1) * 512],
                                    vecT[k][:],
                                    Wk_sb[k][:, col0:col0 + 512],
                                    start=False, stop=(k == KCH - 1),
                                )

                # activations (ACT): sigmoid(i), tanh(g), sigmoid(f), sigmoid(o)
                i_s = wp.tile([BL, H_DIM], f32, tag="i_s", name="i_s")
                g_t = wp.tile([BL, H_DIM], f32, tag="g_t", name="g_t")
                f_s = wp.tile([BL, H_DIM], f32, tag="f_s", name="f_s")
                o_s = wp.tile([BL, H_DIM], f32, tag="o_s", name="o_s")
                nc.scalar.activation(i_s[:], gtiles["i"][:], AF.Sigmoid)
                nc.scalar.activation(g_t[:], gtiles["g"][:], AF.Tanh)
                nc.scalar.activation(f_s[:], gtiles["f"][:], AF.Sigmoid)
                nc.scalar.activation(o_s[:], gtiles["o"][:], AF.Sigmoid)

                # cell math (DVE, fp32)
                t2 = wp.tile([BL, H_DIM], f32, tag="t2", name="t2")
                nc.vector.tensor_mul(out=t2[:], in0=i_s[:], in1=g_t[:])
                t1 = wp.tile([BL, H_DIM], f32, tag="t1", name="t1")
                nc.vector.tensor_mul(out=t1[:], in0=f_s[:], in1=cst[rd][:])
                nc.vector.tensor_add(out=cst[wr][:], in0=t1[:], in1=t2[:])
                tc_t = wp.tile([BL, H_DIM], f32, tag="tc_t", name="tc_t")
                nc.scalar.activation(tc_t[:], cst[wr][:], AF.Tanh)
                h_new = wp.tile([BL, H_DIM], f32, tag="h_new", name="h_new")
                nc.vector.tensor_mul(out=h_new[:], in0=o_s[:], in1=tc_t[:])

                # transpose h_new -> hT (bf16) via PE
                for k in range(8):
                    pt = tp.tile([128, BL], f32, tag="tpsum", name="tpsum")
                    nc.tensor.transpose(pt[:], h_new[:, k * 128:(k + 1) * 128], ident[:])
                    nc.any.tensor_copy(out=hT[wr][k][:], in_=pt[:])

                # z matmul (batch-major): z_pre[64, 128] = h_new @ W_fc.T + b_fc
                zps = zp.tile([BL, O_DIM], f32, tag="zpsum", name="zpsum")
                nc.tensor.matmul(zps[:], ones_sb[:], bfc_sb[:], start=True, stop=False)
                for k in range(8):
                    nc.tensor.matmul(zps[:], hT[wr][k][:], WfcT_sb[k][:],
                                     start=False, stop=(k == 7))
                z_sb = wp.tile([BL, O_DIM], f32, tag="z_sb", name="z_sb")
                nc.scalar.activation(z_sb[:], zps[:], AF.Tanh)
                # mask (per-partition scalar) and stage for output DMA
                nc.vector.tensor_scalar_mul(zstage[:, j, :], z_sb[:], m_tile[:, j:j + 1])
                # transpose masked z -> zT bf16
                zpt = tp.tile([128, BL], f32, tag="tpsum", name="tpsum")
                nc.tensor.transpose(zpt[:], zstage[:, j, :], ident[:])
                nc.any.tensor_copy(out=zT[wr][:], in_=zpt[:])

                final_h[0] = h_new

            def block(t0):
                """U steps starting at t0 (register or int)."""
                x_tile = xp.tile([I_DIM, U, BL], bf16, tag="x", name="x")
                nc.sync.dma_start(x_tile[:], xr[:, ds(t0, U), :])
                m_tile = xp.tile([BL, U], f32, tag="m", name="m")
                nc.sync.dma_start(m_tile[:], mask_d[:, ds(t0, U)])
                zstage = xp.tile([BL, U, O_DIM], f32, tag="zstage", name="zstage")
                for j in range(U):
                    step(j, x_tile[:, j, :], m_tile, zstage)
                nc.sync.dma_start(ozr[:, ds(t0, U), :], zstage[:])

            if T_steps == U:
                block(0)
            else:
                with tc.For_i(0, T_steps, U) as t0:
                    block(t0)

            # final state out (state ends in slot 0 since U is even)
            nc.sync.dma_start(oh_d[:], final_h[0][:])
            nc.sync.dma_start(oc_d[:], cst[0][:])

    nc.compile()
    return nc


def _prep_core_inputs(inputs, core):
    """Host-side prep: shard along batch, transpose/pack for the kernel layout."""
    import ml_dtypes

    bf16 = ml_dtypes.bfloat16
    b0, b1 = core * BL, (core + 1) * BL
    x = np.ascontiguousarray(inputs["inputs"][:, b0:b1, :])          # [T, BL, I]
    xT = np.ascontiguousarray(x.transpose(0, 2, 1)).astype(bf16)     # [T, I, BL]
    h0T = np.ascontiguousarray(inputs["h0"][b0:b1].T).reshape(8, 128, BL).astype(bf16)
    z0T = np.ascontiguousarray(inputs["z0"][b0:b1].T).astype(bf16)   # [128, BL]
    c0 = np.ascontiguousarray(inputs["c0"][b0:b1]).astype(np.float32)
    sl = inputs["sequence_lengths"][b0:b1]
    T = x.shape[0]
    masks = (np.arange(T)[None, :] < sl[:, None]).astype(np.float32)  # [BL, T]
    W_full = np.concatenate([inputs["W_ih"], inputs["W_hh"]], axis=1)  # [4096, 1280]
    Wk = np.ascontiguousarray(W_full.T).reshape(KCH, 128, G).astype(bf16)
    bias = (inputs["b_ih"] + inputs["b_hh"]).reshape(1, G).astype(bf16)
    WfcT = np.ascontiguousarray(inputs["W_fc"].T).reshape(8, 128, O_DIM).astype(bf16)
    bfc = inputs["b_fc"].reshape(1, O_DIM).astype(bf16)
    return {
        "xT": xT, "h0T": h0T, "z0T": z0T, "c0": c0, "masks": masks,
        "Wk": Wk, "bias": bias, "WfcT": WfcT, "bfc": bfc,
    }


def _get_nc(T_steps, U):
    key = (T_steps, U)
    if key not in _cache:
        _cache[key] = _build(T_steps, U)
    return _cache[key]


def run(inputs, T_steps=T_FULL, U=8, trace=False):
    from concourse.bass_utils import run_bass_kernel_spmd

    nc = _get_nc(T_steps, U)
    if T_steps != T_FULL:
        inputs = dict(inputs)
        inputs["inputs"] = inputs["inputs"][:T_steps]
    in_maps = [_prep_core_inputs(inputs, c) for c in range(NCORES)]
    res = run_bass_kernel_spmd(nc, in_maps, list(range(NCORES)), trace=trace)
    outs = res.results
    T = T_steps
    out_z = np.concatenate([outs[c]["out_z"] for c in range(NCORES)], axis=1)
    out_h = np.concatenate([outs[c]["out_h"] for c in range(NCORES)], axis=0)
    out_c = np.concatenate([outs[c]["out_c"] for c in range(NCORES)], axis=0)
    return (out_z, out_h, out_c), res


def kernel(**inputs):
    inputs = {k: np.asarray(v) for k, v in inputs.items()}
    (out_z, out_h, out_c), _ = run(inputs)
    return (out_z.astype(np.float32), out_h.astype(np.float32),
            out_c.astype(np.float32))
